# revision 11
# baseline (speedup 1.0000x reference)
"""Trainium2 Bass kernel for a rate-1/2, constraint-length-3 feedforward
convolutional encoder (generator polynomials "101" and "111", MSB-first).

The trellis scan in the reference collapses to elementwise XORs of shifted
input bits (zero initial state):

    out0[t] = u[t] ^ u[t-2]            (poly "101")
    out1[t] = u[t] ^ u[t-1] ^ u[t-2]   (poly "111")

with the codeword interleaved time-major: y[:, 2t] = out0[t], y[:, 2t+1] = out1[t].

The kernel is pure HBM traffic, so the device works on uint8 tensors (the
bits are 0/1 — exact in u8; the host casts at the numpy boundary): 6.25 MiB
of device traffic per core instead of 24 MiB in f32.

Layout: block-transposed, slot-major. SBUF partition p holds an 18-slot
window u[16p-2 .. 16p+16) of all 1024 codewords of the core (slot k is a
contiguous 1024-byte run holding bit u[16p-2+k] of every codeword). The
u[t-1] / u[t-2] shifts become slot offsets (multiples of 1024 bytes), so
every XOR runs full-width on uint32 lanes (4 codewords per lane-cycle) with
no partition-offset or byte-misaligned access. The 2-slot overlap between
consecutive partitions (+12.5% input re-read) replaces any cross-partition
dependency; the encoder's zero initial state is two host-padded zero slots
in partition 0.

Raw-Bass implementation (no TileContext): the tile framework costs ~2us of
start-up handshakes and ~4us of end-of-kernel per-semaphore clears, which
dominate a ~17us DMA-bound kernel. Here the whole dependency graph is five
semaphores: three input-DMA completions (SP HWDGE ring), one Vector
progress counter gating the output DMAs (GpSimd SWDGE ring), and one
output-DMA completion counter. Slot groups are uneven (2/4/4/4/2) so the
first XOR starts as soon as the first 512 KiB of input lands and the first
output DMA fires ~1.3us later.

Sharding: pure data parallel over the batch dim across 8 NeuronCores.
"""

import numpy as np

N_CORES = 8
B, K = 8192, 2048
N_OUT = 2
SHARD_B = B // N_CORES          # 1024 codewords per core
W = SHARD_B // 4                # 256 uint32 words per slot (4 codewords each)
P = 128                         # SBUF partitions
SLOTS = K // P                  # 16 output slots per partition
IN_SLOTS = SLOTS + 2            # +2 overlap slots for the u[t-1]/u[t-2] taps

# Uneven slot groups: starts and sizes (output slots per group).
G_START = [0, 2, 6, 10, 14]
G_SIZE = [2, 4, 4, 4, 2]
# Input sub-DMA slot boundaries; group g's inputs [k0, k0+gs+2) determine
# which sub-DMAs it waits on.
IN_BOUNDS = [0, 4, 11, IN_SLOTS]

_compiled = {}


def _build_nc():
    import concourse.bass as bass  # noqa: F401
    from concourse import bacc, mybir

    nc = bacc.Bacc(
        "TRN2",
        target_bir_lowering=False,
        debug=False,
        enable_asserts=False,
    )
    # x row p = 18 slots x 1024 codeword-bits: u[16p-2+k][b] at word k*W + b/4
    x = nc.dram_tensor(
        "x", [P, IN_SLOTS * W], mybir.dt.uint32, kind="ExternalInput"
    ).ap()
    # y row p = [j, k, b]: bit j of symbol t=16p+k -> word j*SLOTS*W + k*W + b/4
    y = nc.dram_tensor(
        "y", [P, N_OUT * SLOTS * W], mybir.dt.uint32, kind="ExternalOutput"
    ).ap()

    xin = nc.alloc_sbuf_tensor("xin", [P, IN_SLOTS * W], mybir.dt.uint32).ap()
    out = nc.alloc_sbuf_tensor("out", [P, N_OUT * SLOTS * W], mybir.dt.uint32).ap()

    in_sems = [nc.alloc_semaphore(f"in_sem{i}") for i in range(3)]
    d_sem = nc.alloc_semaphore("d_sem")
    v_sem = nc.alloc_semaphore("v_sem")
    out_sem = nc.alloc_semaphore("out_sem")

    xor = mybir.AluOpType.bitwise_xor
    n_groups = len(G_START)

    # Last input sub-DMA each group must wait for (group input slots are
    # [k0, k0+gs+2), sub-DMA i covers [IN_BOUNDS[i], IN_BOUNDS[i+1])).
    def last_dma(g):
        hi = G_START[g] + G_SIZE[g] + 2  # exclusive
        for i in range(3):
            if IN_BOUNDS[i + 1] >= hi:
                return i
        raise AssertionError

    with nc.Block(no_gpsimd_drain=True) as blk:

        @blk.sync
        def _(eng):
            for i in range(3):
                s0, s1 = IN_BOUNDS[i], IN_BOUNDS[i + 1]
                eng.dma_start(
                    xin[:, s0 * W : s1 * W], x[:, s0 * W : s1 * W]
                ).then_inc(in_sems[i], 16)

        @blk.vector
        def _(eng):
            waited = -1
            for g in range(n_groups):
                need = last_dma(g)
                while waited < need:
                    waited += 1
                    eng.wait_ge(in_sems[waited], 16)
                k0, gs = G_START[g], G_SIZE[g]
                a = xin[:, (k0 + 2) * W : (k0 + 2 + gs) * W]  # u[t]
                b = xin[:, (k0 + 1) * W : (k0 + 1 + gs) * W]  # u[t-1]
                c = xin[:, k0 * W : (k0 + gs) * W]            # u[t-2]
                out0 = out[:, k0 * W : (k0 + gs) * W]
                out1 = out[:, (SLOTS + k0) * W : (SLOTS + k0 + gs) * W]
                # DVE pipelines consecutive instructions (queue depth 8), so
                # the out1 XOR must explicitly wait for its out0 input.
                eng.tensor_tensor(out0, a, c, xor).then_inc(d_sem, 1)
                eng.wait_ge(d_sem, g + 1)
                eng.tensor_tensor(out1, out0, b, xor).then_inc(v_sem, 1)

        @blk.gpsimd
        def _(eng):
            for g in range(n_groups):
                eng.wait_ge(v_sem, g + 1)
                k0, gs = G_START[g], G_SIZE[g]
                for j in range(N_OUT):
                    cols = slice((j * SLOTS + k0) * W, (j * SLOTS + k0 + gs) * W)
                    eng.dma_start(y[:, cols], out[:, cols]).then_inc(out_sem, 16)
            eng.wait_ge(out_sem, 2 * n_groups * 16)

    # Reset semaphores for the next execution of the NEFF — after the
    # end-of-block all-engine barrier, so no update can race the clear.
    for s in (*in_sems, d_sem, v_sem, out_sem):
        nc.sync.sem_clear(s)

    nc.compile()
    return nc


def _get_nc():
    if "nc" not in _compiled:
        _compiled["nc"] = _build_nc()
    return _compiled["nc"]


def _shard_inputs(x_full: np.ndarray) -> list[dict]:
    """Cast the 0/1 float input to uint8 and build the per-core block-
    transposed, slot-overlapped layout (see module docstring)."""
    xu8 = x_full.astype(np.uint8)            # exact: values are 0.0 / 1.0
    in_maps = []
    for i in range(N_CORES):
        xt = np.ascontiguousarray(xu8[i * SHARD_B : (i + 1) * SHARD_B].T)
        blk = xt.reshape(P, SLOTS, SHARD_B)  # [p, k, b] = u[16p+k][b]
        xb = np.zeros((P, IN_SLOTS, SHARD_B), np.uint8)
        xb[:, 2:] = blk
        xb[1:, :2] = blk[:-1, SLOTS - 2 :]   # u[16p-2], u[16p-1]
        in_maps.append({"x": xb.reshape(P, IN_SLOTS * SHARD_B).view(np.uint32)})
    return in_maps


def _gather_output(results) -> np.ndarray:
    """Un-transpose and interleave: y[p, j, k, b] -> out[b, 2*(16p+k)+j]."""
    out = np.empty((B, N_OUT * K), np.float32)
    for i, r in enumerate(results):
        y_t = r["y"].view(np.uint8).reshape(P, N_OUT, SLOTS, SHARD_B)
        out[i * SHARD_B : (i + 1) * SHARD_B] = (
            y_t.transpose(3, 0, 2, 1).reshape(SHARD_B, N_OUT * K)
        )
    return out


def kernel(**inputs) -> np.ndarray:
    from concourse.bass_utils import run_bass_kernel_spmd

    x_full = np.ascontiguousarray(np.asarray(inputs["inputs"], dtype=np.float32))
    assert x_full.shape == (B, K), x_full.shape

    nc = _get_nc()
    in_maps = _shard_inputs(x_full)
    res = run_bass_kernel_spmd(nc, in_maps, core_ids=list(range(N_CORES)))
    return _gather_output(res.results)


# revision 16
# speedup vs baseline: 1.0067x; 1.0067x over previous
"""Trainium2 Bass kernel for a rate-1/2, constraint-length-3 feedforward
convolutional encoder (generator polynomials "101" and "111", MSB-first).

The trellis scan in the reference collapses to elementwise XORs of shifted
input bits (zero initial state):

    out0[t] = u[t] ^ u[t-2]            (poly "101")
    out1[t] = u[t] ^ u[t-1] ^ u[t-2]   (poly "111")

with the codeword interleaved time-major: y[:, 2t] = out0[t], y[:, 2t+1] = out1[t].

The kernel is pure HBM traffic, so the device works on uint8 tensors (the
bits are 0/1 — exact in u8; the host casts at the numpy boundary): 6.25 MiB
of device traffic per core instead of 24 MiB in f32.

Layout: block-transposed, slot-major. SBUF partition p holds an 18-slot
window u[16p-2 .. 16p+16) of all 1024 codewords of the core (slot k is a
contiguous 1024-byte run holding bit u[16p-2+k] of every codeword). The
u[t-1] / u[t-2] shifts become slot offsets (multiples of 1024 bytes), so
every XOR runs full-width on uint32 lanes (4 codewords per lane-cycle) with
no partition-offset or byte-misaligned access. The 2-slot overlap between
consecutive partitions (+12.5% input re-read) replaces any cross-partition
dependency; the encoder's zero initial state is two host-padded zero slots
in partition 0.

Raw-Bass implementation (no TileContext): the tile framework's start-up
handshakes and end-of-kernel per-semaphore clears are material overhead on
a ~17us DMA-bound kernel, so the dependency graph is hand-built from seven
semaphores: three input-DMA completions, out0/out1 per-group progress
counters from Vector, and two output-DMA completion counters. The input
streams on the SP HWDGE ring; out0 DMAs go to the otherwise-empty GpSimd
SWDGE queue so the first one fires while the input is still in flight, and
out1 DMAs follow the input on the SP queue. Four slot groups of 4 pipeline
compute against both DMA directions.

Sharding: pure data parallel over the batch dim across 8 NeuronCores.
"""

import numpy as np

N_CORES = 8
B, K = 8192, 2048
N_OUT = 2
SHARD_B = B // N_CORES          # 1024 codewords per core
W = SHARD_B // 4                # 256 uint32 words per slot (4 codewords each)
P = 128                         # SBUF partitions
SLOTS = K // P                  # 16 output slots per partition
IN_SLOTS = SLOTS + 2            # +2 overlap slots for the u[t-1]/u[t-2] taps

# Slot groups: starts and sizes (output slots per group).
G_START = [0, 4, 8, 12]
G_SIZE = [4, 4, 4, 4]
# Input sub-DMA slot boundaries; group g's inputs [k0, k0+gs+2) determine
# which sub-DMAs it waits on.
IN_BOUNDS = [0, 6, 13, IN_SLOTS]

_compiled = {}


def _build_nc():
    import concourse.bass as bass  # noqa: F401
    from concourse import bacc, mybir

    nc = bacc.Bacc(
        "TRN2",
        target_bir_lowering=False,
        debug=False,
        enable_asserts=False,
    )
    # x row p = 18 slots x 1024 codeword-bits: u[16p-2+k][b] at word k*W + b/4
    x = nc.dram_tensor(
        "x", [P, IN_SLOTS * W], mybir.dt.uint32, kind="ExternalInput"
    ).ap()
    # y row p = [j, k, b]: bit j of symbol t=16p+k -> word j*SLOTS*W + k*W + b/4
    y = nc.dram_tensor(
        "y", [P, N_OUT * SLOTS * W], mybir.dt.uint32, kind="ExternalOutput"
    ).ap()

    xin = nc.alloc_sbuf_tensor("xin", [P, IN_SLOTS * W], mybir.dt.uint32).ap()
    out = nc.alloc_sbuf_tensor("out", [P, N_OUT * SLOTS * W], mybir.dt.uint32).ap()

    in_sems = [nc.alloc_semaphore(f"in_sem{i}") for i in range(3)]
    v0_sem = nc.alloc_semaphore("v0_sem")   # out0 group completions
    v1_sem = nc.alloc_semaphore("v1_sem")   # out1 group completions
    out0_sem = nc.alloc_semaphore("out0_sem")  # SWDGE j0 completions
    out1_sem = nc.alloc_semaphore("out1_sem")  # HWDGE j1 completions

    xor = mybir.AluOpType.bitwise_xor
    n_groups = len(G_START)

    # Last input sub-DMA each group must wait for (group input slots are
    # [k0, k0+gs+2), sub-DMA i covers [IN_BOUNDS[i], IN_BOUNDS[i+1])).
    def last_dma(g):
        hi = G_START[g] + G_SIZE[g] + 2  # exclusive
        for i in range(3):
            if IN_BOUNDS[i + 1] >= hi:
                return i
        raise AssertionError

    def ycols(j, g):
        k0, gs = G_START[g], G_SIZE[g]
        return slice((j * SLOTS + k0) * W, (j * SLOTS + k0 + gs) * W)

    with nc.Block(no_gpsimd_drain=True) as blk:

        @blk.sync
        def _(eng):
            for i in range(3):
                s0, s1 = IN_BOUNDS[i], IN_BOUNDS[i + 1]
                eng.dma_start(
                    xin[:, s0 * W : s1 * W], x[:, s0 * W : s1 * W]
                ).then_inc(in_sems[i], 16)
            # out1 DMAs ride the same HWDGE queue behind the input; by the
            # time out1 of group g is computed the input stream is done.
            for g in range(n_groups):
                eng.wait_ge(v1_sem, g + 1)
                eng.dma_start(y[:, ycols(1, g)], out[:, ycols(1, g)]).then_inc(
                    out1_sem, 16
                )

        @blk.vector
        def _(eng):
            waited = -1
            for g in range(n_groups):
                need = last_dma(g)
                while waited < need:
                    waited += 1
                    eng.wait_ge(in_sems[waited], 16)
                k0, gs = G_START[g], G_SIZE[g]
                a = xin[:, (k0 + 2) * W : (k0 + 2 + gs) * W]  # u[t]
                b = xin[:, (k0 + 1) * W : (k0 + 1 + gs) * W]  # u[t-1]
                c = xin[:, k0 * W : (k0 + gs) * W]            # u[t-2]
                out0 = out[:, k0 * W : (k0 + gs) * W]
                out1 = out[:, (SLOTS + k0) * W : (SLOTS + k0 + gs) * W]
                eng.tensor_tensor(out0, a, c, xor).then_inc(v0_sem, 1)
                # DVE pipelines consecutive instructions (queue depth 8), so
                # the out1 XOR must explicitly wait for its out0 input.
                eng.wait_ge(v0_sem, g + 1)
                eng.tensor_tensor(out1, out0, b, xor).then_inc(v1_sem, 1)

        @blk.gpsimd
        def _(eng):
            # out0 DMAs on the (empty) SWDGE queue: the first fires as soon
            # as out0 of group 0 is computed, while the input is in flight.
            for g in range(n_groups):
                eng.wait_ge(v0_sem, g + 1)
                eng.dma_start(y[:, ycols(0, g)], out[:, ycols(0, g)]).then_inc(
                    out0_sem, 16
                )
            eng.wait_ge(out0_sem, n_groups * 16)
            eng.wait_ge(out1_sem, n_groups * 16)

    # Reset semaphores for the next execution of the NEFF — after the
    # end-of-block all-engine barrier, so no update can race the clear.
    for s in (*in_sems, v0_sem, v1_sem, out0_sem, out1_sem):
        nc.sync.sem_clear(s)

    nc.compile()
    return nc


def _get_nc():
    if "nc" not in _compiled:
        _compiled["nc"] = _build_nc()
    return _compiled["nc"]


def _shard_inputs(x_full: np.ndarray) -> list[dict]:
    """Cast the 0/1 float input to uint8 and build the per-core block-
    transposed, slot-overlapped layout (see module docstring)."""
    xu8 = x_full.astype(np.uint8)            # exact: values are 0.0 / 1.0
    in_maps = []
    for i in range(N_CORES):
        xt = np.ascontiguousarray(xu8[i * SHARD_B : (i + 1) * SHARD_B].T)
        blk = xt.reshape(P, SLOTS, SHARD_B)  # [p, k, b] = u[16p+k][b]
        xb = np.zeros((P, IN_SLOTS, SHARD_B), np.uint8)
        xb[:, 2:] = blk
        xb[1:, :2] = blk[:-1, SLOTS - 2 :]   # u[16p-2], u[16p-1]
        in_maps.append({"x": xb.reshape(P, IN_SLOTS * SHARD_B).view(np.uint32)})
    return in_maps


def _gather_output(results) -> np.ndarray:
    """Un-transpose and interleave: y[p, j, k, b] -> out[b, 2*(16p+k)+j]."""
    out = np.empty((B, N_OUT * K), np.float32)
    for i, r in enumerate(results):
        y_t = r["y"].view(np.uint8).reshape(P, N_OUT, SLOTS, SHARD_B)
        out[i * SHARD_B : (i + 1) * SHARD_B] = (
            y_t.transpose(3, 0, 2, 1).reshape(SHARD_B, N_OUT * K)
        )
    return out


def kernel(**inputs) -> np.ndarray:
    from concourse.bass_utils import run_bass_kernel_spmd

    x_full = np.ascontiguousarray(np.asarray(inputs["inputs"], dtype=np.float32))
    assert x_full.shape == (B, K), x_full.shape

    nc = _get_nc()
    in_maps = _shard_inputs(x_full)
    res = run_bass_kernel_spmd(nc, in_maps, core_ids=list(range(N_CORES)))
    return _gather_output(res.results)


# revision 17
# speedup vs baseline: 1.0486x; 1.0415x over previous
"""Trainium2 Bass kernel for a rate-1/2, constraint-length-3 feedforward
convolutional encoder (generator polynomials "101" and "111", MSB-first).

The trellis scan in the reference collapses to elementwise XORs of shifted
input bits (zero initial state):

    out0[t] = u[t] ^ u[t-2]            (poly "101")
    out1[t] = u[t] ^ u[t-1] ^ u[t-2]   (poly "111")

with the codeword interleaved time-major: y[:, 2t] = out0[t], y[:, 2t+1] = out1[t].

The kernel is pure HBM traffic, so the device works on uint8 tensors (the
bits are 0/1 — exact in u8; the host casts at the numpy boundary): 6.25 MiB
of device traffic per core instead of 24 MiB in f32.

Layout: block-transposed, slot-major. SBUF partition p holds an 18-slot
window u[16p-2 .. 16p+16) of all 1024 codewords of the core (slot k is a
contiguous 1024-byte run holding bit u[16p-2+k] of every codeword). The
u[t-1] / u[t-2] shifts become slot offsets (multiples of 1024 bytes), so
every XOR runs full-width on uint32 lanes — 4 codewords per lane-cycle —
with no partition-offset or byte-misaligned access. The 2-slot overlap
between consecutive partitions (+12.5% input re-read) replaces any
cross-partition dependency; the encoder's zero initial state is two
host-padded zero slots in partition 0.

Sharding: pure data parallel over the batch dim across 8 NeuronCores.
"""

import numpy as np

N_CORES = 8
B, K = 8192, 2048
N_OUT = 2
SHARD_B = B // N_CORES          # 1024 codewords per core
W = SHARD_B // 4                # 256 uint32 words per slot (4 codewords each)
P = 128                         # SBUF partitions
SLOTS = K // P                  # 16 output slots per partition
IN_SLOTS = SLOTS + 2            # +2 overlap slots for the u[t-1]/u[t-2] taps
GROUPS = 4                      # slot groups for pipelining
GS = SLOTS // GROUPS            # 4 output slots per group

_compiled = {}


def _build_nc():
    import concourse.bass as bass  # noqa: F401
    import concourse.tile as tile
    from concourse import bacc, mybir

    nc = bacc.Bacc(
        "TRN2",
        target_bir_lowering=False,
        debug=False,
        enable_asserts=False,
    )
    # x row p = 18 slots x 1024 codeword-bits: u[16p-2+k][b] at word k*W + b/4
    x = nc.dram_tensor(
        "x", [P, IN_SLOTS * W], mybir.dt.uint32, kind="ExternalInput"
    ).ap()
    # y row p = [j, k, b]: bit j of symbol t=16p+k -> word j*SLOTS*W + k*W + b/4
    y = nc.dram_tensor(
        "y", [P, N_OUT * SLOTS * W], mybir.dt.uint32, kind="ExternalOutput"
    ).ap()

    xor = mybir.AluOpType.bitwise_xor

    with tile.TileContext(nc) as tc:
        with (
            tc.tile_pool(name="xin", bufs=1) as in_pool,
            tc.tile_pool(name="out", bufs=1) as out_pool,
        ):
            xin = in_pool.tile([P, IN_SLOTS * W], mybir.dt.uint32, tag="xin", name="xin")
            out = out_pool.tile(
                [P, N_OUT * SLOTS * W], mybir.dt.uint32, tag="out", name="out"
            )

            # Input sub-DMAs on the SP HWDGE ring: slot ranges [0,6), [6,12),
            # [12,18). Compute group g needs input slots [4g, 4g+6), so group
            # 0 waits only on the first sub-DMA (768 KiB, ~2us) and the XOR
            # stream starts while the rest of the input is still in flight.
            bounds = [0, GS + 2, 2 * GS + 2 + GS // 2, IN_SLOTS]
            for i in range(len(bounds) - 1):
                s0, s1 = bounds[i], bounds[i + 1]
                nc.sync.dma_start(xin[:, s0 * W : s1 * W], x[:, s0 * W : s1 * W])

            for g in range(GROUPS):
                k0 = GS * g
                a = xin[:, (k0 + 2) * W : (k0 + 2 + GS) * W]  # u[t]
                b = xin[:, (k0 + 1) * W : (k0 + 1 + GS) * W]  # u[t-1]
                c = xin[:, k0 * W : (k0 + GS) * W]            # u[t-2]
                out0 = out[:, k0 * W : (k0 + GS) * W]
                out1 = out[:, (SLOTS + k0) * W : (SLOTS + k0 + GS) * W]

                nc.vector.tensor_tensor(out0, a, c, xor)
                nc.vector.tensor_tensor(out1, out0, b, xor)

                # Output DMAs on the SWDGE path (GpSimd sequencer) so input
                # and output streams trigger independently.
                nc.gpsimd.dma_start(y[:, k0 * W : (k0 + GS) * W], out0)
                nc.gpsimd.dma_start(
                    y[:, (SLOTS + k0) * W : (SLOTS + k0 + GS) * W], out1
                )

    nc.compile()
    return nc


def _get_nc():
    if "nc" not in _compiled:
        _compiled["nc"] = _build_nc()
    return _compiled["nc"]


def _shard_inputs(x_full: np.ndarray) -> list[dict]:
    """Cast the 0/1 float input to uint8 and build the per-core block-
    transposed, slot-overlapped layout (see module docstring)."""
    xu8 = x_full.astype(np.uint8)            # exact: values are 0.0 / 1.0
    in_maps = []
    for i in range(N_CORES):
        xt = np.ascontiguousarray(xu8[i * SHARD_B : (i + 1) * SHARD_B].T)
        blk = xt.reshape(P, SLOTS, SHARD_B)  # [p, k, b] = u[16p+k][b]
        xb = np.zeros((P, IN_SLOTS, SHARD_B), np.uint8)
        xb[:, 2:] = blk
        xb[1:, :2] = blk[:-1, SLOTS - 2 :]   # u[16p-2], u[16p-1]
        in_maps.append({"x": xb.reshape(P, IN_SLOTS * SHARD_B).view(np.uint32)})
    return in_maps


def _gather_output(results) -> np.ndarray:
    """Un-transpose and interleave: y[p, j, k, b] -> out[b, 2*(16p+k)+j]."""
    out = np.empty((B, N_OUT * K), np.float32)
    for i, r in enumerate(results):
        y_t = r["y"].view(np.uint8).reshape(P, N_OUT, SLOTS, SHARD_B)
        out[i * SHARD_B : (i + 1) * SHARD_B] = (
            y_t.transpose(3, 0, 2, 1).reshape(SHARD_B, N_OUT * K)
        )
    return out


def kernel(**inputs) -> np.ndarray:
    from concourse.bass_utils import run_bass_kernel_spmd

    x_full = np.ascontiguousarray(np.asarray(inputs["inputs"], dtype=np.float32))
    assert x_full.shape == (B, K), x_full.shape

    nc = _get_nc()
    in_maps = _shard_inputs(x_full)
    res = run_bass_kernel_spmd(nc, in_maps, core_ids=list(range(N_CORES)))
    return _gather_output(res.results)


# revision 18
# speedup vs baseline: 1.0525x; 1.0038x over previous
"""Trainium2 Bass kernel for a rate-1/2, constraint-length-3 feedforward
convolutional encoder (generator polynomials "101" and "111", MSB-first).

The trellis scan in the reference collapses to elementwise XORs of shifted
input bits (zero initial state):

    out0[t] = u[t] ^ u[t-2]            (poly "101")
    out1[t] = u[t] ^ u[t-1] ^ u[t-2]   (poly "111")

with the codeword interleaved time-major: y[:, 2t] = out0[t], y[:, 2t+1] = out1[t].

The kernel is pure HBM traffic, so the device works on uint8 tensors (the
bits are 0/1 — exact in u8; the host casts at the numpy boundary): 6.25 MiB
of device traffic per core instead of 24 MiB in f32.

Layout: block-transposed, slot-major. SBUF partition p holds an 18-slot
window u[16p-2 .. 16p+16) of all 1024 codewords of the core (slot k is a
contiguous 1024-byte run holding bit u[16p-2+k] of every codeword). The
u[t-1] / u[t-2] shifts become slot offsets (multiples of 1024 bytes), so
every XOR runs full-width on uint32 lanes — 4 codewords per lane-cycle —
with no partition-offset or byte-misaligned access. The 2-slot overlap
between consecutive partitions (+12.5% input re-read) replaces any
cross-partition dependency; the encoder's zero initial state is two
host-padded zero slots in partition 0.

Sharding: pure data parallel over the batch dim across 8 NeuronCores.
"""

import numpy as np

N_CORES = 8
B, K = 8192, 2048
N_OUT = 2
SHARD_B = B // N_CORES          # 1024 codewords per core
W = SHARD_B // 4                # 256 uint32 words per slot (4 codewords each)
P = 128                         # SBUF partitions
SLOTS = K // P                  # 16 output slots per partition
IN_SLOTS = SLOTS + 2            # +2 overlap slots for the u[t-1]/u[t-2] taps
# Slot-group sizes: two small leading groups so the first output DMA
# fires ~1us earlier (fills the input->output engine handoff), larger
# groups after for fatter DMA descriptors.
G_SIZE = [2, 2, 4, 4, 4]
G_START = [0, 2, 4, 8, 12]
GS = 4                          # input sub-DMA sizing (unchanged bounds)

_compiled = {}


def _build_nc():
    import concourse.bass as bass  # noqa: F401
    import concourse.tile as tile
    from concourse import bacc, mybir

    nc = bacc.Bacc(
        "TRN2",
        target_bir_lowering=False,
        debug=False,
        enable_asserts=False,
    )
    # x row p = 18 slots x 1024 codeword-bits: u[16p-2+k][b] at word k*W + b/4
    x = nc.dram_tensor(
        "x", [P, IN_SLOTS * W], mybir.dt.uint32, kind="ExternalInput"
    ).ap()
    # y row p = [j, k, b]: bit j of symbol t=16p+k -> word j*SLOTS*W + k*W + b/4
    y = nc.dram_tensor(
        "y", [P, N_OUT * SLOTS * W], mybir.dt.uint32, kind="ExternalOutput"
    ).ap()

    xor = mybir.AluOpType.bitwise_xor

    with tile.TileContext(nc) as tc:
        with (
            tc.tile_pool(name="xin", bufs=1) as in_pool,
            tc.tile_pool(name="out", bufs=1) as out_pool,
        ):
            xin = in_pool.tile([P, IN_SLOTS * W], mybir.dt.uint32, tag="xin", name="xin")
            out = out_pool.tile(
                [P, N_OUT * SLOTS * W], mybir.dt.uint32, tag="out", name="out"
            )

            # Input sub-DMAs on the SP HWDGE ring: slot ranges [0,6), [6,12),
            # [12,18). Compute group g needs input slots [4g, 4g+6), so group
            # 0 waits only on the first sub-DMA (768 KiB, ~2us) and the XOR
            # stream starts while the rest of the input is still in flight.
            bounds = [0, GS + 2, 2 * GS + 2 + GS // 2, IN_SLOTS]
            for i in range(len(bounds) - 1):
                s0, s1 = bounds[i], bounds[i + 1]
                nc.sync.dma_start(xin[:, s0 * W : s1 * W], x[:, s0 * W : s1 * W])

            for k0, gs in zip(G_START, G_SIZE):
                a = xin[:, (k0 + 2) * W : (k0 + 2 + gs) * W]  # u[t]
                b = xin[:, (k0 + 1) * W : (k0 + 1 + gs) * W]  # u[t-1]
                c = xin[:, k0 * W : (k0 + gs) * W]            # u[t-2]
                out0 = out[:, k0 * W : (k0 + gs) * W]
                out1 = out[:, (SLOTS + k0) * W : (SLOTS + k0 + gs) * W]

                nc.vector.tensor_tensor(out0, a, c, xor)
                nc.vector.tensor_tensor(out1, out0, b, xor)

                # Output DMAs on the SWDGE path (GpSimd sequencer) so input
                # and output streams trigger independently.
                nc.gpsimd.dma_start(y[:, k0 * W : (k0 + gs) * W], out0)
                nc.gpsimd.dma_start(
                    y[:, (SLOTS + k0) * W : (SLOTS + k0 + gs) * W], out1
                )

    nc.compile()
    return nc


def _get_nc():
    if "nc" not in _compiled:
        _compiled["nc"] = _build_nc()
    return _compiled["nc"]


def _shard_inputs(x_full: np.ndarray) -> list[dict]:
    """Cast the 0/1 float input to uint8 and build the per-core block-
    transposed, slot-overlapped layout (see module docstring)."""
    xu8 = x_full.astype(np.uint8)            # exact: values are 0.0 / 1.0
    in_maps = []
    for i in range(N_CORES):
        xt = np.ascontiguousarray(xu8[i * SHARD_B : (i + 1) * SHARD_B].T)
        blk = xt.reshape(P, SLOTS, SHARD_B)  # [p, k, b] = u[16p+k][b]
        xb = np.zeros((P, IN_SLOTS, SHARD_B), np.uint8)
        xb[:, 2:] = blk
        xb[1:, :2] = blk[:-1, SLOTS - 2 :]   # u[16p-2], u[16p-1]
        in_maps.append({"x": xb.reshape(P, IN_SLOTS * SHARD_B).view(np.uint32)})
    return in_maps


def _gather_output(results) -> np.ndarray:
    """Un-transpose and interleave: y[p, j, k, b] -> out[b, 2*(16p+k)+j]."""
    out = np.empty((B, N_OUT * K), np.float32)
    for i, r in enumerate(results):
        y_t = r["y"].view(np.uint8).reshape(P, N_OUT, SLOTS, SHARD_B)
        out[i * SHARD_B : (i + 1) * SHARD_B] = (
            y_t.transpose(3, 0, 2, 1).reshape(SHARD_B, N_OUT * K)
        )
    return out


def kernel(**inputs) -> np.ndarray:
    from concourse.bass_utils import run_bass_kernel_spmd

    x_full = np.ascontiguousarray(np.asarray(inputs["inputs"], dtype=np.float32))
    assert x_full.shape == (B, K), x_full.shape

    nc = _get_nc()
    in_maps = _shard_inputs(x_full)
    res = run_bass_kernel_spmd(nc, in_maps, core_ids=list(range(N_CORES)))
    return _gather_output(res.results)
